# revision 5
# baseline (speedup 1.0000x reference)
"""DySample (scale=2, groups=4) Trainium2 Bass kernel.

Contract: kernel(**inputs) takes the FULL inputs from setup_inputs() and
returns the FULL output (8, 16, 256, 256) f32. Internally shards
data-parallel over batch: core b computes batch element b.

Algorithm (per core, one batch element):
  The reference pipeline (offset 1x1 conv -> coords -> pixel_shuffle ->
  grid_sample(border) -> end 1x1 conv) collapses to:
    - sample position for fine pixel (2h+i, 2w+j), group g:
        ix = w + u_x,  iy = h + u_y,  u = init_pos + 0.25*conv(x)  (|u|<0.5)
    - bilinear+border == 3-tap tent in each axis; since |u - init_pos| << 0.25
      only the 2 taps {w+j-1, w+j} x {h+i-1, h+i} are nonzero, with weights
      linear in u (no floor/select needed); border clamp == edge-replicated
      shifts (weights still sum to 1).
    - the end conv (C=64 -> 16) commutes with sampling per group, so it is
      applied FIRST at coarse resolution (block-diag matmul), and sampling
      runs on the 16 conv-ed channels per group, accumulating over groups.
  One fused PE pass computes both the end conv and the offset conv as a
  [65 x 96] matmul per coarse column (65 = 64 ch + bias row).
"""

import os
import sys

for _p in ("/opt/trn_rl_repo", "/root/.axon_site/_ro/trn_rl_repo"):
    if os.path.isdir(_p) and _p not in sys.path:
        sys.path.append(_p)

import numpy as np

import concourse.bass as bass
import concourse.mybir as mb
import concourse.tile as tile
from concourse.bass_utils import run_bass_kernel_spmd
from concourse.tile import TileContext
from concourse.vector_clock import ScopedClock

B, C, H, W = 8, 64, 128, 128
G, S = 4, 2
CP = 132  # padded w-pitch of xe tiles (2 left, 2 right)
F16 = mb.dt.float16
F32 = mb.dt.float32

# ---------------------------------------------------------------------------
# Toolchain workarounds (this container's walrus rejects >1 sem wait per
# instruction, and any sem-ge wait on a Drain).
# ---------------------------------------------------------------------------


def _patched_drain_and_barrier(self, tick_clock, wait_clock):
    d = self.nc.sync.drain()
    wait_clock.add_sem_waits(d.ins, ScopedClock({None: tick_clock.global_clock}))
    waits = list(d.ins.sync_info.on_wait or [])
    d.ins.sync_info.on_wait = []
    by_num = {h.num: h for h in self.sems.allocated().values()}
    for w in waits:
        assert w.wait_mode == "sem-ge-imm" and w.wait_reg is None, w
        self.nc.sync.wait_ge(by_num[w.id], w.wait_value)

    self.nc.all_engine_barrier()
    assert self.sems is not None
    popped = self.nc._tile_sem_poison_stack.pop()
    assert popped is self._sem_poison
    self.nc.clear_and_free_semaphores(list(self.sems.allocated().values()))
    self.nc.all_engine_barrier()


def _split_multiwait_bir(bir_json: bytes) -> bytes:
    import json

    j = json.loads(bir_json)
    ctr = 0
    for fn in j["functions"]:
        for bb in fn["blocks"]:
            out = []
            changed = False
            for inst in bb["instructions"]:
                si = inst.get("sync_info")
                waits = si.get("on_wait") if si else None
                if waits:
                    if inst.get("opcode") == "Drain":
                        keep = [w for w in waits if w.get("wait_mode") == "sem-eq-imm"]
                    else:
                        keep = waits[-1:]
                    hoist = [w for w in waits if w not in keep]
                    if hoist:
                        changed = True
                        for w in hoist:
                            ctr += 1
                            out.append(
                                {
                                    "debug": inst.get("debug", 10),
                                    "engine": inst["engine"],
                                    "ins": [],
                                    "name": f"WSPLIT-{ctr}",
                                    "opcode": "EventSemaphore",
                                    "outs": [],
                                    "sync_info": {"on_update": [], "on_wait": [w]},
                                }
                            )
                        si["on_wait"] = keep
                out.append(inst)
            if changed:
                bb["instructions"] = out
    return json.dumps(j).encode()


_patched = False


def _apply_patches():
    global _patched
    if _patched:
        return
    _patched = True
    tile.TileContext._drain_and_barrier = _patched_drain_and_barrier

    import concourse.bass2jax as bass2jax
    import concourse.bass_utils as bass_utils

    orig = bass_utils.compile_bir_kernel

    def patched_compile(bir_json, tmpdir, neff_name="file.neff"):
        return orig(_split_multiwait_bir(bir_json), tmpdir, neff_name)

    bass2jax.compile_bir_kernel = patched_compile
    bass_utils.compile_bir_kernel = patched_compile


# ---------------------------------------------------------------------------
# Host-side weight prep
# ---------------------------------------------------------------------------


def _init_pos() -> np.ndarray:
    # mirrors reference._init_pos: (2, G*s, s) -> 32 channels
    s, g = S, G
    h = (np.arange(s, dtype=np.float32) - (s - 1) / 2) / s
    m0, m1 = np.meshgrid(h, h, indexing="ij")
    ip = np.stack([m0, m1]).transpose(0, 2, 1)  # (2, s, s)
    ip = np.tile(ip, (1, g, 1))  # (2, G*s, s)
    return ip.reshape(32).astype(np.float32)


def _host_weights(offset_w, offset_b, end_w, end_b) -> np.ndarray:
    wcomb = np.zeros((65, 96), np.float32)
    for g in range(G):
        sl = slice(g * 16, (g + 1) * 16)
        wcomb[sl, sl] = end_w[:, sl].T  # [c_in, o] block
        wcomb[64, sl] = end_b / 4.0
    wcomb[0:64, 64:96] = 0.25 * offset_w.T
    wcomb[64, 64:96] = 0.25 * offset_b + _init_pos()
    return wcomb


# ---------------------------------------------------------------------------
# Device kernel
# ---------------------------------------------------------------------------


def _build_nc(debug: bool = False) -> bass.Bass:
    nc = bass.Bass("TRN2", target_bir_lowering=False, debug=False, num_devices=8)
    xin = nc.dram_tensor("xin", [65, H * W], F32, kind="ExternalInput")
    wcomb = nc.dram_tensor("wcomb", [65, 96], F32, kind="ExternalInput")
    out = nc.dram_tensor("out", [16, 2 * H, 2 * W], F32, kind="ExternalOutput")
    if debug:
        dbg = {
            "xe": nc.dram_tensor("dbg_xe", [128, 64 * CP], F16, kind="ExternalOutput"),
            "xm": nc.dram_tensor("dbg_xm", [128, 64 * CP], F16, kind="ExternalOutput"),
            "u": nc.dram_tensor("dbg_u", [128, 32 * 128], F16, kind="ExternalOutput"),
            "xu": nc.dram_tensor("dbg_xu", [128, 64 * CP], F16, kind="ExternalOutput"),
            "xum": nc.dram_tensor("dbg_xum", [128, 64 * CP], F16, kind="ExternalOutput"),
            "xd": nc.dram_tensor("dbg_xd", [128, 64 * CP], F16, kind="ExternalOutput"),
            "xdm": nc.dram_tensor("dbg_xdm", [128, 64 * CP], F16, kind="ExternalOutput"),
            "P0": nc.dram_tensor("dbg_P0", [128, 16 * 128], F16, kind="ExternalOutput"),
            "P1": nc.dram_tensor("dbg_P1", [128, 16 * 128], F16, kind="ExternalOutput"),
            "P2": nc.dram_tensor("dbg_P2", [128, 16 * 128], F16, kind="ExternalOutput"),
            "P3": nc.dram_tensor("dbg_P3", [128, 16 * 128], F16, kind="ExternalOutput"),
            "z": nc.dram_tensor("dbg_z", [128, 16 * 16 * 64], F16, kind="ExternalOutput"),
        }

    mult, add = mb.AluOpType.mult, mb.AluOpType.add

    with TileContext(nc) as tc:
        with (
            tc.tile_pool(name="const", bufs=1) as pc,
            tc.tile_pool(name="main", bufs=1) as pm,
        ):
            wsb = pc.tile([65, 96], F32)
            nc.sync.dma_start(wsb[:], wcomb[:])

            xe = pm.tile([128, 64 * CP], F16, tag="xe")
            xm = pm.tile([128, 64 * CP], F16, tag="xm")
            u = pm.tile([128, 32 * 128], F16, tag="u")

            # ---------------- phase A: fused conv matmuls ----------------
            with (
                tc.tile_pool(name="xp", bufs=1) as px,
                tc.tile_pool(name="ps", bufs=3, space="PSUM") as pp,
            ):
                xext = px.tile([65, H * W], F32)
                nc.sync.dma_start(xext[:], xin[:])
                xv = xext[:].rearrange("p (h w) -> p h w", w=W)  # [65, 128, 128]
                xe_v = xe[:].rearrange("p (c w) -> p c w", w=CP)
                xm_v = xm[:].rearrange("p (c w) -> p c w", w=CP)
                u_v = u[:].rearrange("p (c w) -> p c w", w=128)
                WCHUNK = 8
                for ch in range(W // WCHUNK):
                    # slot pitch 128 (bank divisor) so no matmul crosses a bank
                    ps = pp.tile([128, WCHUNK * 128], F32)
                    for wi in range(WCHUNK):
                        w0 = ch * WCHUNK + wi
                        nc.tensor.matmul(
                            ps[:, wi * 128 : wi * 128 + 96],
                            xv[:, :, w0],  # lhsT [65, 128]
                            wsb[:],  # rhs  [65, 96]
                            start=True,
                            stop=True,
                        )
                    pv = ps[:].rearrange("p (w c) -> p c w", c=128)  # [128, 128, 8]
                    c0 = ch * WCHUNK
                    nc.scalar.copy(xe_v[:, :, 2 + c0 : 2 + c0 + WCHUNK], pv[:, 0:64, :])
                    nc.scalar.copy(xm_v[:, :, 3 + c0 : 3 + c0 + WCHUNK], pv[:, 0:64, :])
                    nc.scalar.copy(u_v[:, :, c0 : c0 + WCHUNK], pv[:, 64:96, :])

                # edge fixups (border replicate):
                # xm positions: [3+w] -> pos2 (w=-1 -> xe[0]), pos131 (w+2=129 -> xe[127])
                nc.scalar.copy(xm_v[:, :, 2:3], xe_v[:, :, 2:3])
                nc.scalar.copy(xm_v[:, :, 131:132], xe_v[:, :, 129:130])
                # dead pad columns (never read, but the variant DMAs copy whole
                # rows): zero so nothing is uninitialized
                nc.vector.memset(xe_v[:, :, 0:2], 0.0)
                nc.vector.memset(xe_v[:, :, 130:132], 0.0)
                nc.vector.memset(xm_v[:, :, 0:2], 0.0)

            # ---------------- phase B ----------------
            with tc.tile_pool(name="pb", bufs=1) as pb:
                # h-shifted variants (replicate row 0 / row 127)
                xu = pb.tile([128, 64 * CP], F16, tag="xu")
                xum = pb.tile([128, 64 * CP], F16, tag="xum")
                xd = pb.tile([128, 64 * CP], F16, tag="xd")
                xdm = pb.tile([128, 64 * CP], F16, tag="xdm")
                nc.sync.dma_start(xu[1:128, :], xe[0:127, :])
                nc.sync.dma_start(xu[0:1, :], xe[0:1, :])
                nc.sync.dma_start(xum[1:128, :], xm[0:127, :])
                nc.sync.dma_start(xum[0:1, :], xm[0:1, :])
                nc.sync.dma_start(xd[0:127, :], xe[1:128, :])
                nc.sync.dma_start(xd[127:128, :], xe[127:128, :])
                nc.sync.dma_start(xdm[0:127, :], xm[1:128, :])
                nc.sync.dma_start(xdm[127:128, :], xm[127:128, :])

                # tap-weight images: per slot (g, i, j):
                #   vx0 = j==0 ? -ux : 1-ux      vx1 = j==0 ? 1+ux : ux
                #   vy0 = i==0 ? -uy : 1-uy      vy1 = i==0 ? 1+uy : uy
                vx = [pb.tile([128, 16 * 128], F16, name=f"vx{b}", tag=f"vx{b}") for b in range(2)]
                vy = [pb.tile([128, 16 * 128], F16, name=f"vy{a}", tag=f"vy{a}") for a in range(2)]
                uu = u[:].rearrange(
                    "p (xy g i j w) -> p xy g i j w", xy=2, g=4, i=2, j=2, w=128
                )
                for t in range(2):
                    xv_ = vx[t][:].rearrange("p (g i j w) -> p g i j w", g=4, i=2, w=128)
                    yv_ = vy[t][:].rearrange("p (g i j w) -> p g i j w", g=4, i=2, w=128)
                    for sub in range(2):
                        s1 = -1.0 if t == 0 else 1.0
                        s2 = float(t ^ sub)  # t0: sub0->0 sub1->1 ; t1: sub0->1 sub1->0
                        nc.vector.tensor_scalar(
                            xv_[:, :, :, sub, :], uu[:, 0, :, :, sub, :], s1, s2, mult, add
                        )
                        nc.vector.tensor_scalar(
                            yv_[:, :, sub, :, :], uu[:, 1, :, sub, :, :], s1, s2, mult, add
                        )

                # corner weights P[a*2+b] = vy_a * vx_b, all 16 (g,i,j) slots
                P = [pb.tile([128, 16 * 128], F16, name=f"P{k}", tag=f"P{k}") for k in range(4)]
                for a in range(2):
                    for b in range(2):
                        nc.vector.tensor_tensor(
                            P[a * 2 + b][:], vy[a][:], vx[b][:], mult
                        )
                if debug:
                    for nm, t in (("xe", xe), ("xm", xm), ("u", u), ("xu", xu),
                                  ("xum", xum), ("xd", xd), ("xdm", xdm),
                                  ("P0", P[0]), ("P1", P[1]), ("P2", P[2]), ("P3", P[3])):
                        nc.sync.dma_start(dbg[nm][:], t[:])

                z = pb.tile([128, 16 * 16 * 64], F16, tag="z")
                ostage = pb.tile([128, 16 * 256], F32, tag="ostage")
                zv = z[:].rearrange("p (s o w) -> p s o w", o=16, w=64)
                ov = ostage[:].rearrange("p (o w two) -> p o w two", w=128, two=2)
                variants = {  # (dh, use_m) -> tile
                    (-1, 0): xu, (-1, 1): xum, (0, 0): xe, (0, 1): xm,
                    (1, 0): xd, (1, 1): xdm,
                }
                for i in range(2):
                    for j in range(2):
                        for half in range(2):
                            w0 = half * 64
                            for g in range(G):
                                gij = g * 4 + i * 2 + j
                                for a in range(2):
                                    for b in range(2):
                                        dh = i - 1 + a
                                        dw = j - 1 + b
                                        vt = variants[(dh, 1 if dw else 0)]
                                        woff = 2 + (2 if dw == 1 else 0) + w0
                                        src = vt[:].rearrange(
                                            "p (c w) -> p c w", w=CP
                                        )[:, g * 16 : g * 16 + 16, woff : woff + 64]
                                        pw = P[a * 2 + b][:].rearrange(
                                            "p (s w) -> p s w", w=128
                                        )[:, gij : gij + 1, w0 : w0 + 64].broadcast_to(
                                            (128, 16, 64)
                                        )
                                        nc.vector.tensor_tensor(
                                            zv[:, g * 4 + a * 2 + b], pw, src, mult
                                        )
                            # sum 16 slots (tree), last level writes interleaved f32
                            zf = z[:]
                            nc.vector.tensor_tensor(
                                zf[:, 0:8192], zf[:, 0:8192], zf[:, 8192:16384], add
                            )
                            nc.vector.tensor_tensor(
                                zf[:, 0:4096], zf[:, 0:4096], zf[:, 4096:8192], add
                            )
                            nc.vector.tensor_tensor(
                                zf[:, 0:2048], zf[:, 0:2048], zf[:, 2048:4096], add
                            )
                            dst = ov[:, :, w0 : w0 + 64, j]
                            nc.vector.tensor_tensor(
                                dst, zv[:, 0], zv[:, 1], add
                            )
                    if debug and i == 0:
                        nc.sync.dma_start(dbg["z"][:], z[:])
                    # emit rows r = 2h+i
                    dv = out[:].rearrange(
                        "o (h two) q -> h o two q", two=2
                    )[:, :, i, :]
                    sv = ostage[:].rearrange("p (o q) -> p o q", q=256)
                    nc.sync.dma_start(dv, sv)

    return nc


_NC = None


def _get_nc():
    global _NC
    if _NC is None:
        _apply_patches()
        _NC = _build_nc()
    return _NC


def _prep_inputs(x, offset_w, offset_b, end_w, end_b):
    x = np.asarray(x, np.float32)
    wcomb = _host_weights(
        np.asarray(offset_w, np.float32),
        np.asarray(offset_b, np.float32),
        np.asarray(end_w, np.float32),
        np.asarray(end_b, np.float32),
    )
    in_maps = []
    for b in range(B):
        xb = np.concatenate(
            [x[b].reshape(64, H * W), np.ones((1, H * W), np.float32)], axis=0
        )
        in_maps.append({"xin": xb, "wcomb": wcomb})
    return in_maps


def run(x, offset_w, offset_b, end_w, end_b, trace=False):
    nc = _get_nc()
    in_maps = _prep_inputs(x, offset_w, offset_b, end_w, end_b)
    res = run_bass_kernel_spmd(nc, in_maps, list(range(B)), trace=trace)
    out = np.stack([res.results[b]["out"] for b in range(B)])
    return out, res


def kernel(x, offset_w, offset_b, end_w, end_b):
    out, _ = run(x, offset_w, offset_b, end_w, end_b)
    return out


# revision 24
# speedup vs baseline: 2.3538x; 2.3538x over previous
"""DySample (scale=2, groups=4) Trainium2 Bass kernel.

Contract: kernel(**inputs) takes the FULL inputs from setup_inputs() and
returns the FULL output (8, 16, 256, 256) f32. Internally shards
data-parallel over batch: core b computes batch element b.

Algorithm (per core, one batch element):
  The reference pipeline (offset 1x1 conv -> coords -> pixel_shuffle ->
  grid_sample(border) -> end 1x1 conv) collapses to:
    - sample position for fine pixel (2h+i, 2w+j), group g:
        ix = w + u_x,  iy = h + u_y,  u = init_pos + 0.25*conv(x)  (|u|<0.5)
    - bilinear+border == 3-tap tent in each axis; since |u - init_pos| << 0.25
      only the 2 taps {w+j-1, w+j} x {h+i-1, h+i} are nonzero, with weights
      linear in u (no floor/select needed); border clamp == edge-replicated
      shifts (weights still sum to 1).
    - the end conv (C=64 -> 16) commutes with sampling per group, so it is
      applied FIRST at coarse resolution (block-diag matmul), and sampling
      runs on the 16 conv-ed channels per group, accumulating over groups.
  One fused PE pass computes both the end conv and the offset conv as a
  [65 x 96] matmul per coarse column (65 = 64 ch + bias row).
"""

import os
import sys

for _p in ("/opt/trn_rl_repo", "/root/.axon_site/_ro/trn_rl_repo"):
    if os.path.isdir(_p) and _p not in sys.path:
        sys.path.append(_p)

import numpy as np

import concourse.bass as bass
import concourse.mybir as mb
import concourse.tile as tile
from concourse.bass_utils import run_bass_kernel_spmd
from concourse.tile import TileContext
from concourse.vector_clock import ScopedClock

B, C, H, W = 8, 64, 128, 128
G, S = 4, 2
CP = 132  # padded w-pitch of xe tiles (2 left, 2 right)
F16 = mb.dt.float16
F32 = mb.dt.float32

# ---------------------------------------------------------------------------
# Toolchain workarounds (this container's walrus rejects >1 sem wait per
# instruction, and any sem-ge wait on a Drain).
# ---------------------------------------------------------------------------


def _patched_drain_and_barrier(self, tick_clock, wait_clock):
    d = self.nc.sync.drain()
    wait_clock.add_sem_waits(d.ins, ScopedClock({None: tick_clock.global_clock}))
    waits = list(d.ins.sync_info.on_wait or [])
    d.ins.sync_info.on_wait = []
    by_num = {h.num: h for h in self.sems.allocated().values()}
    for w in waits:
        assert w.wait_mode == "sem-ge-imm" and w.wait_reg is None, w
        self.nc.sync.wait_ge(by_num[w.id], w.wait_value)

    self.nc.all_engine_barrier()
    assert self.sems is not None
    popped = self.nc._tile_sem_poison_stack.pop()
    assert popped is self._sem_poison
    self.nc.clear_and_free_semaphores(list(self.sems.allocated().values()))
    self.nc.all_engine_barrier()


def _split_multiwait_bir(bir_json: bytes) -> bytes:
    import json

    j = json.loads(bir_json)
    ctr = 0
    for fn in j["functions"]:
        for bb in fn["blocks"]:
            out = []
            changed = False
            for inst in bb["instructions"]:
                si = inst.get("sync_info")
                waits = si.get("on_wait") if si else None
                if waits:
                    if inst.get("opcode") == "Drain":
                        keep = [w for w in waits if w.get("wait_mode") == "sem-eq-imm"]
                    else:
                        keep = waits[-1:]
                    hoist = [w for w in waits if w not in keep]
                    if hoist:
                        changed = True
                        for w in hoist:
                            ctr += 1
                            out.append(
                                {
                                    "debug": inst.get("debug", 10),
                                    "engine": inst["engine"],
                                    "ins": [],
                                    "name": f"WSPLIT-{ctr}",
                                    "opcode": "EventSemaphore",
                                    "outs": [],
                                    "sync_info": {"on_update": [], "on_wait": [w]},
                                }
                            )
                        si["on_wait"] = keep
                out.append(inst)
            if changed:
                bb["instructions"] = out
    return json.dumps(j).encode()


_patched = False


def _apply_patches():
    global _patched
    if _patched:
        return
    _patched = True
    tile.TileContext._drain_and_barrier = _patched_drain_and_barrier

    import concourse.bass2jax as bass2jax
    import concourse.bass_utils as bass_utils

    orig = bass_utils.compile_bir_kernel

    def patched_compile(bir_json, tmpdir, neff_name="file.neff"):
        return orig(_split_multiwait_bir(bir_json), tmpdir, neff_name)

    bass2jax.compile_bir_kernel = patched_compile
    bass_utils.compile_bir_kernel = patched_compile


# ---------------------------------------------------------------------------
# Host-side weight prep
# ---------------------------------------------------------------------------


def _init_pos() -> np.ndarray:
    # mirrors reference._init_pos: (2, G*s, s) -> 32 channels
    s, g = S, G
    h = (np.arange(s, dtype=np.float32) - (s - 1) / 2) / s
    m0, m1 = np.meshgrid(h, h, indexing="ij")
    ip = np.stack([m0, m1]).transpose(0, 2, 1)  # (2, s, s)
    ip = np.tile(ip, (1, g, 1))  # (2, G*s, s)
    return ip.reshape(32).astype(np.float32)


def _host_weights(offset_w, offset_b, end_w, end_b) -> np.ndarray:
    wcomb = np.zeros((65, 96), np.float32)
    for g in range(G):
        sl = slice(g * 16, (g + 1) * 16)
        wcomb[sl, sl] = end_w[:, sl].T  # [c_in, o] block
        wcomb[64, sl] = end_b / 4.0
    wcomb[0:64, 64:96] = 0.25 * offset_w.T
    wcomb[64, 64:96] = 0.25 * offset_b + _init_pos()
    return wcomb


# ---------------------------------------------------------------------------
# Device kernel
# ---------------------------------------------------------------------------


def _build_nc(debug: bool = False) -> bass.Bass:
    nc = bass.Bass("TRN2", target_bir_lowering=False, debug=False, num_devices=8)
    xin = nc.dram_tensor("xin", [65, H * W], F16, kind="ExternalInput")
    wcomb = nc.dram_tensor("wcomb", [65, 96], F16, kind="ExternalInput")
    shifts = nc.dram_tensor("shifts", [128, 256], F16, kind="ExternalInput")
    out = nc.dram_tensor("out", [16, 2 * H, 2 * W], F32, kind="ExternalOutput")
    if debug:
        dbg = {
            "xe": nc.dram_tensor("dbg_xe", [128, 64 * CP], F16, kind="ExternalOutput"),
            "xm": nc.dram_tensor("dbg_xm", [128, 64 * CP], F16, kind="ExternalOutput"),
            "u": nc.dram_tensor("dbg_u", [128, 32 * 128], F16, kind="ExternalOutput"),
            "xu": nc.dram_tensor("dbg_xu", [128, 64 * CP], F16, kind="ExternalOutput"),
            "xum": nc.dram_tensor("dbg_xum", [128, 64 * CP], F16, kind="ExternalOutput"),
            "xd": nc.dram_tensor("dbg_xd", [128, 64 * CP], F16, kind="ExternalOutput"),
            "xdm": nc.dram_tensor("dbg_xdm", [128, 64 * CP], F16, kind="ExternalOutput"),
            "P0": nc.dram_tensor("dbg_P0", [128, 16 * 128], F16, kind="ExternalOutput"),
            "P1": nc.dram_tensor("dbg_P1", [128, 16 * 128], F16, kind="ExternalOutput"),
            "P2": nc.dram_tensor("dbg_P2", [128, 16 * 128], F16, kind="ExternalOutput"),
            "P3": nc.dram_tensor("dbg_P3", [128, 16 * 128], F16, kind="ExternalOutput"),
            "z": nc.dram_tensor("dbg_z", [128, 16 * 16 * 64], F16, kind="ExternalOutput"),
        }

    mult, add = mb.AluOpType.mult, mb.AluOpType.add

    with TileContext(nc) as tc:
        with (
            tc.tile_pool(name="const", bufs=1) as pc,
            tc.tile_pool(name="main", bufs=1) as pm,
        ):
            wsb = pc.tile([65, 96], F16)
            nc.sync.dma_start(wsb[:], wcomb[:])
            ssb = pc.tile([128, 256], F16)
            nc.sync.dma_start(ssb[:], shifts[:])

            xe = pm.tile([128, 64 * CP], F16, tag="xe")
            xm = pm.tile([128, 64 * CP], F16, tag="xm")
            u = pm.tile([128, 32 * 128], F16, tag="u")
            xu = pm.tile([128, 64 * CP], F16, tag="xu")
            xum = pm.tile([128, 64 * CP], F16, tag="xum")
            xd = pm.tile([128, 64 * CP], F16, tag="xd")
            xdm = pm.tile([128, 64 * CP], F16, tag="xdm")

            xe_v = xe[:].rearrange("p (c w) -> p c w", w=CP)
            xm_v = xm[:].rearrange("p (c w) -> p c w", w=CP)
            u_v = u[:].rearrange("p (c w) -> p c w", w=128)

            if debug:
                for t in (xe, xu, xd):
                    tv = t[:].rearrange("p (c w) -> p c w", w=CP)
                    nc.vector.memset(tv[:, :, 0:2], 0.0)
                    nc.vector.memset(tv[:, :, 130:132], 0.0)
                for t in (xm, xum, xdm):
                    tv = t[:].rearrange("p (c w) -> p c w", w=CP)
                    nc.vector.memset(tv[:, :, 0:2], 0.0)

            # ---------------- phase A: fused conv + shifted variants -------
            # Per 8-column chunk: conv matmuls (stationary = x column) land in
            # psum pixel-major; evictions split across ACT and DVE (DVE is
            # otherwise idle here). PE h-shift matmuls (stationary = 0/1 shift
            # matrices) build xu/xd, double-evicted (+0, +1) for the w-shifted
            # m-variants.
            with (
                tc.tile_pool(name="xp", bufs=1) as px,
                tc.tile_pool(name="ps", bufs=3, space="PSUM") as pp,
                tc.tile_pool(name="ps2", bufs=1, space="PSUM") as pp2,
            ):
                xext = px.tile([65, H * W], F16)
                nc.sync.dma_start(xext[:], xin[:])
                xv = xext[:].rearrange("p (h w) -> p h w", w=W)  # [65, 128, 128]
                WCHUNK = 8
                for ch in range(W // WCHUNK):
                    c0 = ch * WCHUNK
                    # slot pitch 128 (bank divisor) so no matmul crosses a bank
                    ps = pp.tile([128, WCHUNK * 128], F32)
                    for wi in range(WCHUNK):
                        nc.tensor.matmul(
                            ps[:, wi * 128 : wi * 128 + 96],
                            xv[:, :, c0 + wi],  # lhsT [65, 128]
                            wsb[:],  # rhs  [65, 96]
                            start=True,
                            stop=True,
                        )
                    pv = ps[:].rearrange("p (w c) -> p c w", c=128)  # [128,128,8]
                    nc.scalar.copy(xe_v[:, :, 2 + c0 : 2 + c0 + WCHUNK], pv[:, 0:64, :])
                    nc.scalar.copy(xm_v[:, :, 3 + c0 : 3 + c0 + WCHUNK], pv[:, 0:64, :])
                    nc.vector.tensor_copy(u_v[:, :, c0 : c0 + WCHUNK], pv[:, 64:96, :])
                    if ch == 0:
                        # border-replicate fixup: xm pos2 == xe[w=0]
                        nc.scalar.copy(xm_v[:, :, 2:3], xe_v[:, :, 2:3])

                    ps2 = pp2.tile([128, 1024], F32)
                    src = xe_v[:, :, 2 + c0 : 2 + c0 + WCHUNK]  # [128, 64, 8]
                    nc.tensor.matmul(
                        ps2[:, 0:512], ssb[:, 0:128], src, start=True, stop=True
                    )
                    nc.tensor.matmul(
                        ps2[:, 512:1024], ssb[:, 128:256], src, start=True, stop=True
                    )
                    for slot, t0, t1, ea, eb in (
                        (0, xu, xum, nc.scalar, nc.vector),
                        (512, xd, xdm, nc.scalar, nc.vector),
                    ):
                        pv2 = ps2[:, slot : slot + 512].rearrange(
                            "p (c w) -> p c w", w=WCHUNK
                        )
                        t0v = t0[:].rearrange("p (c w) -> p c w", w=CP)
                        t1v = t1[:].rearrange("p (c w) -> p c w", w=CP)
                        ea.copy(t0v[:, :, 2 + c0 : 2 + c0 + WCHUNK], pv2)
                        eb.tensor_copy(t1v[:, :, 3 + c0 : 3 + c0 + WCHUNK], pv2)
                    if ch == 0:
                        for tv, t0 in ((xum, xu), (xdm, xd)):
                            nc.scalar.copy(
                                tv[:].rearrange("p (c w) -> p c w", w=CP)[:, :, 2:3],
                                t0[:].rearrange("p (c w) -> p c w", w=CP)[:, :, 2:3],
                            )
                    if ch == W // WCHUNK - 1:
                        nc.scalar.copy(xm_v[:, :, 131:132], xe_v[:, :, 129:130])
                        for tv, t0 in ((xum, xu), (xdm, xd)):
                            nc.scalar.copy(
                                tv[:].rearrange("p (c w) -> p c w", w=CP)[:, :, 131:132],
                                t0[:].rearrange("p (c w) -> p c w", w=CP)[:, :, 129:130],
                            )

            # ---------------- phase B: sampling ----------------
            with tc.tile_pool(name="pb", bufs=1) as pb:
                if debug:
                    for nm, t in (("xe", xe), ("xm", xm), ("u", u), ("xu", xu),
                                  ("xum", xum), ("xd", xd), ("xdm", xdm)):
                        nc.sync.dma_start(dbg[nm][:], t[:])

                z = pb.tile([128, 16 * 16 * 64], F16, tag="z")
                ost = [
                    pb.tile([128, 16 * 256], F32, name=f"ost{i}", tag=f"ost{i}")
                    for i in range(2)
                ]
                variants = {  # (dh, use_m) -> tile
                    (-1, 0): xu, (-1, 1): xum, (0, 0): xe, (0, 1): xm,
                    (1, 0): xd, (1, 1): xdm,
                }
                uu = u[:].rearrange(
                    "p (xy g i j w) -> p xy g i j w", xy=2, g=4, i=2, j=2, w=128
                )
                for half in range(2):
                    w0 = half * 64
                    # tap weights for this half, per slot (g, i, j):
                    #   vx0 = j==0 ? -ux : 1-ux      vx1 = j==0 ? 1+ux : ux
                    #   vy0 = i==0 ? -uy : 1-uy      vy1 = i==0 ? 1+uy : uy
                    vx = [pb.tile([128, 16 * 64], F16, name=f"vx{b}", tag=f"vx{b}")
                          for b in range(2)]
                    vy = [pb.tile([128, 16 * 64], F16, name=f"vy{a}", tag=f"vy{a}")
                          for a in range(2)]
                    for t in range(2):
                        xv_ = vx[t][:].rearrange("p (g i j w) -> p g i j w", g=4, i=2, w=64)
                        yv_ = vy[t][:].rearrange("p (g i j w) -> p g i j w", g=4, i=2, w=64)
                        for sub in range(2):
                            s1 = -1.0 if t == 0 else 1.0
                            s2 = float(t ^ sub)
                            nc.vector.tensor_scalar(
                                xv_[:, :, :, sub, :], uu[:, 0, :, :, sub, w0 : w0 + 64],
                                s1, s2, mult, add,
                            )
                            nc.vector.tensor_scalar(
                                yv_[:, :, sub, :, :], uu[:, 1, :, sub, :, w0 : w0 + 64],
                                s1, s2, mult, add,
                            )
                    P = [pb.tile([128, 16 * 64], F16, name=f"P{k}", tag=f"P{k}")
                         for k in range(4)]
                    for a in range(2):
                        for b in range(2):
                            nc.vector.tensor_tensor(P[a * 2 + b][:], vy[a][:], vx[b][:], mult)

                    for i in range(2):
                        for j in range(2):
                            for a in range(2):
                                for b in range(2):
                                    dh = i - 1 + a
                                    dw = j - 1 + b
                                    vt = variants[(dh, 1 if dw else 0)]
                                    woff = 2 + (2 if dw == 1 else 0) + w0
                                    src = vt[:].rearrange(
                                        "p (g o w) -> p g o w", g=4, o=16, w=CP
                                    )[:, :, :, woff : woff + 64]
                                    pw = (
                                        P[a * 2 + b][:]
                                        .rearrange("p (g c w) -> p g c w", g=4, c=4, w=64)
                                        [:, :, i * 2 + j]
                                        .unsqueeze(2)
                                        .broadcast_to((128, 4, 16, 64))
                                    )
                                    dst = z[:].rearrange(
                                        "p (g s o w) -> p g s o w", g=4, s=4, o=16, w=64
                                    )[:, :, a * 2 + b]
                                    nc.vector.tensor_tensor(dst, pw, src, mult)
                            # sum 16 slots (tree); last level writes f32 interleaved
                            zf = z[:]
                            nc.vector.tensor_tensor(
                                zf[:, 0:8192], zf[:, 0:8192], zf[:, 8192:16384], add
                            )
                            nc.vector.tensor_tensor(
                                zf[:, 0:4096], zf[:, 0:4096], zf[:, 4096:8192], add
                            )
                            nc.vector.tensor_tensor(
                                zf[:, 0:2048], zf[:, 0:2048], zf[:, 2048:4096], add
                            )
                            ov = ost[i][:].rearrange(
                                "p (o w two) -> p o w two", w=128, two=2
                            )
                            zvv = z[:].rearrange("p (s o w) -> p s o w", o=16, w=64)
                            # final add at fp16 2x on DVE; f32 strided convert
                            # lands on ACT (idle in this phase)
                            l4 = pb.tile([128, 1024], F16, name="l4", tag="l4", bufs=2)
                            nc.vector.tensor_tensor(l4[:], zvv[:, 0], zvv[:, 1], add)
                            nc.scalar.copy(
                                ov[:, :, w0 : w0 + 64, j],
                                l4[:].rearrange("p (o w) -> p o w", w=64),
                            )
                        if half == 1:
                            # ost[i] is complete after its (half=1, j=1) round;
                            # emit its DMA now so it overlaps later compute
                            dv = out[:].rearrange(
                                "o (h two) q -> h o two q", two=2
                            )[:, :, i, :]
                            sv = ost[i][:].rearrange("p (o q) -> p o q", q=256)
                            nc.sync.dma_start(dv, sv)
                    if half == 1 and debug:
                        nc.sync.dma_start(dbg["z"][:], z[:])

    return nc


_NC = None


def _get_nc():
    global _NC
    if _NC is None:
        _apply_patches()
        _NC = _build_nc()
    return _NC


def _shift_mats() -> np.ndarray:
    s = np.zeros((128, 256), np.float16)
    for m in range(128):
        s[max(m - 1, 0), m] = 1.0  # xu[m] = xe[m-1 clamped]
        s[min(m + 1, 127), 128 + m] = 1.0  # xd[m] = xe[m+1 clamped]
    return s


def _prep_inputs(x, offset_w, offset_b, end_w, end_b):
    x = np.asarray(x, np.float32)
    wcomb = _host_weights(
        np.asarray(offset_w, np.float32),
        np.asarray(offset_b, np.float32),
        np.asarray(end_w, np.float32),
        np.asarray(end_b, np.float32),
    )
    smat = _shift_mats()
    in_maps = []
    for b in range(B):
        xb = np.concatenate(
            [x[b].reshape(64, H * W), np.ones((1, H * W), np.float32)], axis=0
        ).astype(np.float16)
        in_maps.append({"xin": xb, "wcomb": wcomb.astype(np.float16), "shifts": smat})
    return in_maps


def run(x, offset_w, offset_b, end_w, end_b, trace=False):
    nc = _get_nc()
    in_maps = _prep_inputs(x, offset_w, offset_b, end_w, end_b)
    res = run_bass_kernel_spmd(nc, in_maps, list(range(B)), trace=trace)
    out = np.stack([res.results[b]["out"] for b in range(B)])
    return out, res


def kernel(x, offset_w, offset_b, end_w, end_b):
    out, _ = run(x, offset_w, offset_b, end_w, end_b)
    return out
